# revision 19
# baseline (speedup 1.0000x reference)
"""Trainium2 Bass kernel for nn_ChargePredict (segment_reduce).

Sharding: data-parallel over atoms with molecule-aligned shard boundaries so
segment sums stay core-local (one-hot columns zeroed outside each core's own
molecule range; overlap rows discarded on host gather).

The host re-encodes X with a *linear orthonormal* change of basis (same
spirit as the ln_w/ln_b folding): for each (atom, h) the 3x3 block becomes 9
fp16 planes (each 256 h-contiguous)
  [d1,d2,d3, s1,s2,s3, e0,e1, I]
  d_k = (x_ij - x_ji)/sqrt2          (off-diag pairs (0,1),(0,2),(1,2))
  s_k = (x_ij + x_ji)/sqrt2
  e0  = (x00 - x11)/sqrt2,  e1 = (x00 + x11 - 2*x22)/sqrt6
  I   = trace/3
Because (e0, e1) is an orthonormal basis of the traceless-diagonal subspace:
  nA = d1^2+d2^2+d3^2
  nS = s1^2+s2^2+s3^2 + e0^2+e1^2     (no trace correction needed)
  feat = [I, nA, nS] -> LayerNorm -> MLP -> qeq  (identical algebra to ref)
fp16 halves HBM traffic vs fp32 and unlocks DVE 2x modes; squares run mostly
on the Scalar engine, plane sums are 16-bit adds split DVE/GpSimd, LN stats
use bn_stats/bn_aggr, and the LN rsqrt is batched across GK-tile groups so
the ACT table only switches between the silu and sqrt sets twice per group.

Per-core pipeline (atoms on partitions, G=4 blocks of 128 per tile, GK=4
tiles per stats group):
  phase 1 (per tile): DMA planes + I-plane into feat slot; squares in place;
    nA/nS plane adds; bn_stats/bn_aggr
  per group: one Sqrt(var+eps) + reciprocal for GK*G blocks
  phase 2 (per tile): LN apply (TS), PE transposes -> lnT, mm1 fp16 + Silu,
    mm2 fp16 (+b2 via ones-row), out transpose, charges/f^2 (bf16), segment
    matmul with preloaded one-hot blocks
  post: recip(F_u+eps), gather matmuls, batched qeq epilogue
"""

import numpy as np
from contextlib import ExitStack

N_ATOMS = 131072
HID = 256
QD = 16
N_MOL = 1024
LN_EPS = 1e-5
QEQ_EPS = 1e-6

NCORES = 8
MPC = N_MOL // NCORES          # 128 molecules per core
G = 4                          # atom blocks (of 128) per tile
GK = 4                         # tiles per LN-stats group

SQ_DVE = 1                     # planes squared on DVE (rest on ACT)
POOL_ADDS = 0                  # plane-adds offloaded to GpSimd

SQRT2INV = 0.7071067811865476
SQRT6INV = 0.4082482904638631


def _legalize_waits(nc):
    """Walrus codegen accepts at most 1 embedded sync wait per compute
    instruction (2 for DMA). Tile occasionally emits more; split the excess
    onto same-engine ENGINE_NOPs inserted immediately before the offender
    (safe: no reordering, the nop blocks the engine exactly where the wait
    previously lived)."""
    import bass_rust
    eng = {"DVE": nc.vector, "Activation": nc.scalar, "PE": nc.tensor,
           "Pool": nc.gpsimd, "SP": nc.sync}
    f = nc.m.functions[0]
    for blk in f.blocks:
        il = blk.instructions
        idx = 0
        while idx < len(il):
            ins = il[idx]
            cls = ins.__class__.__name__
            si = ins.sync_info
            if cls == "InstEventSemaphore" or not si or not si.on_wait:
                idx += 1
                continue
            limit = 1
            waits = list(si.on_wait)
            if len(waits) <= limit:
                idx += 1
                continue
            engine_name = str(getattr(ins, "engine", "")).split(".")[-1]
            e = eng.get(engine_name, nc.vector)
            excess = waits[:-limit]
            keep = waits[-limit:]
            upd = list(si.on_update) if si.on_update else []
            ins.sync_info = bass_rust.SyncInfo(on_wait=keep, on_update=upd)
            for w in excess:
                nop = e.nop(nofuse=True)
                mi = nop.ins
                for b2 in f.blocks:
                    l2 = b2.instructions
                    for k in range(len(l2) - 1, -1, -1):
                        if l2[k] is mi:
                            del l2[k]
                mi.sync_info = bass_rust.SyncInfo(on_wait=[w], on_update=[])
                il.insert(idx, mi)
                idx += 1
            idx += 1


def _validate_waits(nc):
    f = nc.m.functions[0]
    bad = []
    for blk in f.blocks:
        for ins in blk.instructions:
            if ins.__class__.__name__ == 'InstEventSemaphore':
                continue
            n = (len(ins.sync_info.on_wait)
                 if ins.sync_info and ins.sync_info.on_wait else 0)
            if n > 1:
                bad.append((ins.name, ins.__class__.__name__, n))
    return bad


def _build_program(ncap, variant=0, pool_adds=POOL_ADDS):
    import concourse.bass as bass
    import concourse.tile as tile
    from concourse import mybir

    f32 = mybir.dt.float32
    f16 = mybir.dt.float16
    bf16 = mybir.dt.bfloat16
    AF = mybir.ActivationFunctionType
    OP = mybir.AluOpType
    AX = mybir.AxisListType

    NB = ncap // 128
    NT = NB // G
    NGRP = NT // GK

    nc = bass.Bass("TRN2", target_bir_lowering=False, debug=False,
                   num_devices=NCORES)

    # xk: per-(tile, partition) contiguous planes [G, 8, 256]; xi: I planes
    xk_d = nc.dram_tensor("xk", [NT * 128, G * 2048], f16,
                          kind="ExternalInput").ap()
    xi_d = nc.dram_tensor("xi", [NT * 128, G * 256], f16,
                          kind="ExternalInput").ap()
    qv_d = nc.dram_tensor("qv", [128, NB], bf16, kind="ExternalInput").ap()
    ohn_d = nc.dram_tensor("ohn", [ncap, 128], bf16, kind="ExternalInput").ap()
    oht_d = nc.dram_tensor("oht", [128, ncap], bf16, kind="ExternalInput").ap()
    w1_d = nc.dram_tensor("w1", [128, 1536], f16, kind="ExternalInput").ap()
    b1_d = nc.dram_tensor("b1", [2, 128], f32, kind="ExternalInput").ap()
    w2_d = nc.dram_tensor("w2", [256, 32], f16, kind="ExternalInput").ap()
    b2_d = nc.dram_tensor("b2", [32, 1], f32, kind="ExternalInput").ap()
    id_d = nc.dram_tensor("ident", [128, 128], f16, kind="ExternalInput").ap()
    idb_d = nc.dram_tensor("identb", [32, 32], bf16, kind="ExternalInput").ap()
    out_d = nc.dram_tensor("out", [ncap, QD], f32, kind="ExternalOutput").ap()

    with tile.TileContext(nc) as tc, ExitStack() as ctx:
        singles = ctx.enter_context(tc.tile_pool(name="singles", bufs=1))
        xp = ctx.enter_context(tc.tile_pool(name="xp", bufs=2))
        fp = ctx.enter_context(tc.tile_pool(name="fp", bufs=1))
        sp = ctx.enter_context(tc.tile_pool(name="sp", bufs=2))
        lt = ctx.enter_context(tc.tile_pool(name="lt", bufs=3))
        ps_mm = ctx.enter_context(tc.tile_pool(name="ps_mm", bufs=3, space="PSUM"))
        ps_t = ctx.enter_context(tc.tile_pool(name="ps_t", bufs=3, space="PSUM"))
        ps_seg = ctx.enter_context(tc.tile_pool(name="ps_seg", bufs=1, space="PSUM"))
        big = ctx.enter_context(tc.tile_pool(name="big", bufs=1))

        # ---- constants / weights / one-hots (loaded once) ----
        ident = singles.tile([128, 128], f16)
        nc.scalar.dma_start(out=ident, in_=id_d)
        identb = singles.tile([32, 32], bf16)
        nc.scalar.dma_start(out=identb, in_=idb_d)
        w1_sb = singles.tile([128, 6, 2, 128], f16)
        nc.scalar.dma_start(out=w1_sb,
                          in_=w1_d.rearrange("p (c jb j) -> p c jb j", c=6, jb=2))
        b1_sb = singles.tile([128, 2], f32)
        nc.scalar.dma_start(out=b1_sb, in_=b1_d.rearrange("c p -> p c"))
        w2_sb = singles.tile([128, 2, 32], f16)
        nc.scalar.dma_start(out=w2_sb, in_=w2_d.rearrange("(c p) q -> p c q", p=128))
        b2c = singles.tile([32, 1], f32)
        nc.scalar.dma_start(out=b2c, in_=b2_d)
        qv_sb = singles.tile([128, NB], bf16)
        nc.scalar.dma_start(out=qv_sb, in_=qv_d)
        eps_sb = singles.tile([128, 1], f32)
        nc.vector.memset(eps_sb, LN_EPS)
        dmy = singles.tile([1, 8], bf16)
        nc.vector.memset(dmy, 0.0)
        nc._legalize_dummy = dmy
        ohn_all = singles.tile([128, NB, 128], bf16)
        nc.scalar.dma_start(out=ohn_all,
                            in_=ohn_d.rearrange("(b p) m -> p b m", p=128))
        oht_all = singles.tile([128, NB, 128], bf16)

        # persistent staging across tiles
        cf_st = big.tile([128, NB, 32], bf16)     # [charges | f_u] atom-major
        gath = big.tile([128, NB, 32], bf16)      # gathered [Q_u | recip]
        res = big.tile([128, NB, QD], f32)        # final output staging
        seg_ps = ps_seg.tile([128, 32], f32)      # [Q_u | F_u] per-mol accum

        def phase1(t, fe_tag, ssum, ssq, k):
            xt = xp.tile([128, G, 8, 256], f16, tag="xt")
            nc.sync.dma_start(
                out=xt,
                in_=xk_d[t * 128:(t + 1) * 128].rearrange(
                    "p (g k h) -> p g k h", g=G, h=256))
            fe = fp.tile([128, G, 768], f16, tag=fe_tag)
            nc.sync.dma_start(
                out=fe[:, :, 0:256],
                in_=xi_d[t * 128:(t + 1) * 128].rearrange(
                    "p (g h) -> p g h", g=G))

            # squares in place, split ACT / DVE (GpSimd SBUF traffic stalls
            # concurrent DVE streams ~5x — measured — so Pool stays idle)
            nc.scalar.activation(xt[:, :, 5:8, :], xt[:, :, 5:8, :],
                                 AF.Square)
            nc.vector.tensor_mul(xt[:, :, 0:5, :], xt[:, :, 0:5, :],
                                 xt[:, :, 0:5, :])

            # nA = dd1+dd2+dd3 ; nS = (ss1+ss2+ss3) + (ee0+ee1)
            nA = fe[:, :, 256:512]
            nS = fe[:, :, 512:768]
            scr = sp.tile([128, G, 2, 256], f16, tag="scr")
            nc.vector.tensor_add(nA, xt[:, :, 0, :], xt[:, :, 1, :])
            nc.vector.tensor_add(nA, nA, xt[:, :, 2, :])
            nc.vector.tensor_add(scr[:, :, 0, :], xt[:, :, 3, :], xt[:, :, 4, :])
            nc.vector.tensor_add(scr[:, :, 1, :], xt[:, :, 6, :], xt[:, :, 7, :])
            nc.vector.tensor_add(nS, scr[:, :, 0, :], xt[:, :, 5, :])
            nc.vector.tensor_add(nS, nS, scr[:, :, 1, :])

            # LN stats: Sigma(fe) via pairwise add tree (keeps DVE 2x mode,
            # vs 1x tensor_reduce); Sigma(fe^2) via ACT Square + accumulator
            tr = fp.tile([128, G, 768], f16, tag="tree")
            nc.vector.tensor_add(tr[:, :, 0:384], fe[:, :, 0:384],
                                 fe[:, :, 384:768])
            off, w = 0, 384
            while w > 3:
                h = w // 2
                nc.vector.tensor_add(tr[:, :, off + w:off + w + h],
                                     tr[:, :, off:off + h],
                                     tr[:, :, off + h:off + w])
                off += w
                w = h
            # w == 3 at tr[:, :, off:off+3]
            nc.vector.tensor_add(tr[:, :, off + 3:off + 4],
                                 tr[:, :, off:off + 1],
                                 tr[:, :, off + 1:off + 2])
            nc.vector.tensor_tensor(ssum[:, k], tr[:, :, off + 3:off + 4],
                                    tr[:, :, off + 2:off + 3], OP.add)
            for g in range(G):
                junk = fp.tile([128, 768], f16, tag="junk")
                nc.scalar.activation(junk, fe[:, g, :], AF.Square,
                                     accum_out=ssq[:, k, g:g + 1])
            return fe

        def phase2(t, fe, mu_grp, rstd_grp, k):
            feb = fp.tile([128, G, 768], f16, tag="feb")
            for g in range(G):
                nc.vector.tensor_scalar(feb[:, g, :], fe[:, g, :],
                                        mu_grp[:, k, g:g + 1],
                                        rstd_grp[:, k, g:g + 1],
                                        OP.subtract, OP.mult)

            # transpose ln -> lnT chunks [128f, G*128at]
            lnT = lt.tile([128, 6, G, 128], f16, tag="lnT")
            for cc in range(3):
                tp = ps_t.tile([128, 2, G, 128], f16, tag="tp")
                for ci in range(2):
                    c = 2 * cc + ci
                    for g in range(G):
                        nc.tensor.transpose(
                            tp[:, ci, g, :],
                            feb[:, g, 128 * c:128 * (c + 1)], ident)
                if cc >= 1:
                    nc.scalar.activation(lnT[:, 2 * cc:2 * cc + 2, :, :], tp,
                                         AF.Copy)
                else:
                    nc.vector.tensor_copy(lnT[:, 2 * cc:2 * cc + 2, :, :], tp)

            # mm1 + Silu
            h1T = lt.tile([128, 2, G, 128], f16, tag="h1T")
            for jb in range(2):
                o1 = ps_mm.tile([128, G * 128], f32, tag="mm")
                for c in range(6):
                    nc.tensor.matmul(o1, w1_sb[:, c, jb, :],
                                     lnT[:, c, :, :].rearrange("p g a -> p (g a)"),
                                     start=(c == 0), stop=(c == 5))
                nc.scalar.activation(
                    h1T[:, jb, :, :].rearrange("p g a -> p (g a)"), o1,
                    AF.Silu, bias=b1_sb[:, jb:jb + 1])

            # mm2 (+b2 folded into the PSUM->SBUF copy bias)
            o2 = ps_mm.tile([32, G * 128], f32, tag="mm")
            for c2 in range(2):
                nc.tensor.matmul(o2, w2_sb[:, c2, :],
                                 h1T[:, c2, :, :].rearrange("p g a -> p (g a)"),
                                 start=(c2 == 0), stop=(c2 == 1))
            o2sb = sp.tile([32, G * 128], bf16, tag="o2sb")
            nc.scalar.activation(o2sb, o2, AF.Identity, bias=b2c)

            # atom-major + f_u square + segment accumulate
            pso = ps_t.tile([128, G, 32], bf16, tag="tp")
            for g in range(G):
                nc.tensor.transpose(pso[:, g, :],
                                    o2sb[:, 128 * g:128 * (g + 1)], identb)
            b0 = t * G
            nc.scalar.activation(cf_st[:, b0:b0 + G, 0:16], pso[:, :, 0:16],
                                 AF.Copy)
            nc.scalar.activation(cf_st[:, b0:b0 + G, 16:32], pso[:, :, 16:32],
                                 AF.Square)
            for g in range(G):
                b = b0 + g
                nc.tensor.matmul(seg_ps, ohn_all[:, b, :], cf_st[:, b, :],
                                 start=(b == 0), stop=(b == NB - 1))

        R768 = 1.0 / 768.0
        for tg0 in range(0, NT, GK):
            gksz = min(GK, NT - tg0)
            ssum = sp.tile([128, GK, G, 1], f32, tag="ssum")
            ssq = sp.tile([128, GK, G], f32, tag="ssq")
            mu_grp = sp.tile([128, GK, G], f32, tag="mu")
            rstd_grp = sp.tile([128, GK, G], f32, tag="rstd")
            fes = []
            for k in range(gksz):
                t = tg0 + k
                fes.append(phase1(t, f"fe{t % (GK + 1)}", ssum, ssq, k))
            # mu = ssum/768 ; var = ssq/768 - mu^2 ; rstd = 1/sqrt(var+eps)
            nc.vector.tensor_scalar_mul(mu_grp[:, 0:gksz],
                                        ssum[:, 0:gksz, :, 0], R768)
            nc.vector.tensor_scalar_mul(rstd_grp[:, 0:gksz], ssq[:, 0:gksz],
                                        R768)
            nc.vector.tensor_mul(ssum[:, 0:gksz, :, 0], mu_grp[:, 0:gksz],
                                 mu_grp[:, 0:gksz])
            nc.vector.tensor_tensor(rstd_grp[:, 0:gksz], rstd_grp[:, 0:gksz],
                                    ssum[:, 0:gksz, :, 0], OP.subtract)
            nc.scalar.activation(rstd_grp[:, 0:gksz], rstd_grp[:, 0:gksz],
                                 AF.Sqrt, bias=eps_sb)
            nc.vector.reciprocal(rstd_grp[:, 0:gksz], rstd_grp[:, 0:gksz])
            for k in range(gksz):
                t = tg0 + k
                phase2(t, fes[k], mu_grp, rstd_grp, k)

        # ---- molecule-level post ----
        nc.scalar.dma_start(out=oht_all,
                            in_=oht_d.rearrange("p (b a) -> p b a", a=128))
        mtmp = singles.tile([128, 16], f32)
        nc.vector.tensor_scalar_add(mtmp, seg_ps[:, 16:32], QEQ_EPS)
        nc.vector.reciprocal(mtmp, mtmp)
        mvals = singles.tile([128, 32], bf16)
        nc.vector.tensor_copy(mvals[:, 16:32], mtmp)
        nc.vector.tensor_copy(mvals[:, 0:16], seg_ps[:, 0:16])

        bb = 0
        while bb < NB:
            gw = min(8, NB - bb)
            gp = ps_t.tile([128, 8, 32], f32, tag="tp")
            for j in range(gw):
                nc.tensor.matmul(gp[:, j, :], oht_all[:, bb + j, :], mvals,
                                 start=True, stop=True)
            nc.vector.tensor_copy(gath[:, bb:bb + gw, :], gp[:, 0:gw, :])
            bb += gw

        # ---- batched qeq epilogue ----
        qbc = bass.AP(tensor=qv_sb.tensor, offset=qv_sb.offset,
                      ap=[qv_sb.ap[0], [qv_sb.ap[1][0], NB], [0, QD]])
        # dq = Q - Q_u  (in place over gath Qu slot)
        nc.vector.tensor_tensor(gath[:, :, 0:16], qbc, gath[:, :, 0:16],
                                OP.subtract)
        # scale = f_u * recip (in place over gath recip slot)
        nc.vector.tensor_mul(gath[:, :, 16:32], cf_st[:, :, 16:32],
                             gath[:, :, 16:32])
        corr = xp.tile([128, NB, QD], bf16, tag="xt")
        h = NB // 2
        nc.vector.tensor_mul(corr[:, 0:h], gath[:, 0:h, 0:16],
                             gath[:, 0:h, 16:32])
        nc.vector.tensor_add(res[:, 0:h], cf_st[:, 0:h, 0:16], corr[:, 0:h])
        nc.sync.dma_start(
            out=out_d[0:h * 128].rearrange("(b p) q -> p b q", p=128),
            in_=res[:, 0:h])
        nc.vector.tensor_mul(corr[:, h:NB], gath[:, h:NB, 0:16],
                             gath[:, h:NB, 16:32])
        nc.vector.tensor_add(res[:, h:NB], cf_st[:, h:NB, 0:16],
                             corr[:, h:NB])
        nc.sync.dma_start(
            out=out_d[h * 128:NB * 128].rearrange("(b p) q -> p b q", p=128),
            in_=res[:, h:NB])

    return nc


LAST_EXEC_NS = None


def kernel(X, Q, ln_w, ln_b, W1, b1, W2, b2, batch):
    import ml_dtypes
    from concourse.bass_utils import run_bass_kernel_spmd

    bf = ml_dtypes.bfloat16
    f16 = np.float16
    Xr = np.asarray(X, dtype=np.float32).reshape(N_ATOMS, HID, 9)
    Q = np.asarray(Q, dtype=np.float32)
    batch = np.asarray(batch, dtype=np.int64)

    edges = np.searchsorted(batch, np.arange(0, N_MOL + 1, MPC))
    edges[0] = 0
    edges[-1] = N_ATOMS
    maxcap = int(np.diff(edges).max())
    blk = G * 128
    ncap = max(16896, -(-maxcap // blk) * blk)
    nb = ncap // 128

    # linear orthonormal re-encode: 9 fp16 planes per atom, h-contiguous
    Xp = np.empty((N_ATOMS, 9, HID), dtype=f16)
    Xp[:, 0] = (Xr[:, :, 1] - Xr[:, :, 3]) * SQRT2INV
    Xp[:, 1] = (Xr[:, :, 2] - Xr[:, :, 6]) * SQRT2INV
    Xp[:, 2] = (Xr[:, :, 5] - Xr[:, :, 7]) * SQRT2INV
    Xp[:, 3] = (Xr[:, :, 1] + Xr[:, :, 3]) * SQRT2INV
    Xp[:, 4] = (Xr[:, :, 2] + Xr[:, :, 6]) * SQRT2INV
    Xp[:, 5] = (Xr[:, :, 5] + Xr[:, :, 7]) * SQRT2INV
    Xp[:, 6] = (Xr[:, :, 0] - Xr[:, :, 4]) * SQRT2INV
    Xp[:, 7] = (Xr[:, :, 0] + Xr[:, :, 4] - 2.0 * Xr[:, :, 8]) * SQRT6INV
    Xp[:, 8] = (Xr[:, :, 0] + Xr[:, :, 4] + Xr[:, :, 8]) * (1.0 / 3.0)
    Xp = Xp.reshape(N_ATOMS, 2304)

    ln_w = np.asarray(ln_w, np.float32)
    ln_b = np.asarray(ln_b, np.float32)
    W1 = np.asarray(W1, np.float32)
    W1f = ln_w[:, None] * W1
    b1f = np.asarray(b1, np.float32) + ln_b @ W1
    w1_host = np.ascontiguousarray(
        W1f.reshape(6, 128, 256).transpose(1, 0, 2).reshape(128, 1536)
    ).astype(f16)
    W2h = np.asarray(W2, np.float32).astype(f16)
    b2h = np.ascontiguousarray(
        np.asarray(b2, np.float32).reshape(32, 1))

    nt = nb // G
    in_maps = []
    starts = []
    for c in range(NCORES):
        s, e = int(edges[c]), int(edges[c + 1])
        assert e - s <= ncap, f"core {c} needs {e - s} > {ncap}"
        start = min(s, N_ATOMS - ncap)
        starts.append(start)
        bc = batch[start:start + ncap]
        rel = (bc - c * MPC).astype(np.int64)
        idx = np.arange(ncap) + start
        valid = (idx >= s) & (idx < e) & (rel >= 0) & (rel < MPC)
        ohn = np.zeros((ncap, 128), dtype=np.float32)
        rows = np.nonzero(valid)[0]
        ohn[rows, rel[valid]] = 1.0
        qv = Q[start:start + ncap].reshape(nb, 128).T
        # tile-blocked, partition-contiguous plane/I layout
        v = Xp[start:start + ncap].reshape(nt, G, 128, 2304).transpose(
            0, 2, 1, 3)
        xk = np.ascontiguousarray(v[:, :, :, 0:2048]).reshape(
            nt * 128, G * 2048)
        xi = np.ascontiguousarray(v[:, :, :, 2048:2304]).reshape(
            nt * 128, G * 256)
        in_maps.append({
            "xk": xk,
            "xi": xi,
            "qv": np.ascontiguousarray(qv.astype(bf)),
            "ohn": ohn.astype(bf),
            "oht": np.ascontiguousarray(ohn.T.astype(bf)),
            "w1": w1_host,
            "b1": np.ascontiguousarray(b1f.reshape(2, 128)),
            "w2": W2h,
            "b2": b2h,
            "ident": np.eye(128, dtype=f16),
            "identb": np.eye(32, dtype=bf),
        })

    global LAST_EXEC_NS
    nc = None
    for v in range(2):
        try:
            cand = _build_program(ncap, variant=v)
        except Exception as ex:
            print(f"build variant {v} failed: {ex}")
            continue
        _legalize_waits(cand)
        bad = _validate_waits(cand)
        if not bad:
            nc = cand
            break
        print(f"build variant {v} has over-limit waits: {bad[:3]}")
    assert nc is not None, "no clean build variant found"
    res = run_bass_kernel_spmd(nc, in_maps, core_ids=list(range(NCORES)))
    LAST_EXEC_NS = res.exec_time_ns
    globals()["LAST_RESULT"] = res

    out = np.empty((N_ATOMS, QD), dtype=np.float32)
    for c in range(NCORES):
        s, e = int(edges[c]), int(edges[c + 1])
        r = res.results[c]["out"]
        out[s:e] = r[s - starts[c]:e - starts[c]]
    return out



# revision 24
# speedup vs baseline: 1.0069x; 1.0069x over previous
"""Trainium2 Bass kernel for nn_ChargePredict (segment_reduce).

Sharding: data-parallel over atoms with molecule-aligned shard boundaries so
segment sums stay core-local (one-hot columns zeroed outside each core's own
molecule range; overlap rows discarded on host gather).

The host re-encodes X with a *linear orthonormal* change of basis (same
spirit as the ln_w/ln_b folding): for each (atom, h) the 3x3 block becomes 9
fp16 planes (each 256 h-contiguous)
  [d1,d2,d3, s1,s2,s3, e0,e1, I]
  d_k = (x_ij - x_ji)/sqrt2          (off-diag pairs (0,1),(0,2),(1,2))
  s_k = (x_ij + x_ji)/sqrt2
  e0  = (x00 - x11)/sqrt2,  e1 = (x00 + x11 - 2*x22)/sqrt6
  I   = trace/3
Because (e0, e1) is an orthonormal basis of the traceless-diagonal subspace:
  nA = d1^2+d2^2+d3^2
  nS = s1^2+s2^2+s3^2 + e0^2+e1^2     (no trace correction needed)
  feat = [I, nA, nS] -> LayerNorm -> MLP -> qeq  (identical algebra to ref)
fp16 halves HBM traffic vs fp32 and unlocks DVE 2x modes; squares run mostly
on the Scalar engine, plane sums are 16-bit adds split DVE/GpSimd, LN stats
use bn_stats/bn_aggr, and the LN rsqrt is batched across GK-tile groups so
the ACT table only switches between the silu and sqrt sets twice per group.

Per-core pipeline (atoms on partitions, G=4 blocks of 128 per tile, GK=4
tiles per stats group):
  phase 1 (per tile): DMA planes + I-plane into feat slot; squares in place;
    nA/nS plane adds; bn_stats/bn_aggr
  per group: one Sqrt(var+eps) + reciprocal for GK*G blocks
  phase 2 (per tile): LN apply (TS), PE transposes -> lnT, mm1 fp16 + Silu,
    mm2 fp16 (+b2 via ones-row), out transpose, charges/f^2 (bf16), segment
    matmul with preloaded one-hot blocks
  post: recip(F_u+eps), gather matmuls, batched qeq epilogue
"""

import numpy as np
from contextlib import ExitStack

N_ATOMS = 131072
HID = 256
QD = 16
N_MOL = 1024
LN_EPS = 1e-5
QEQ_EPS = 1e-6

NCORES = 8
MPC = N_MOL // NCORES          # 128 molecules per core
G = 4                          # atom blocks (of 128) per tile
GK = 4                         # tiles per LN-stats group

SQ_DVE = 1                     # planes squared on DVE (rest on ACT)
POOL_ADDS = 0                  # plane-adds offloaded to GpSimd

SQRT2INV = 0.7071067811865476
SQRT6INV = 0.4082482904638631


def _legalize_waits(nc):
    """Walrus codegen accepts at most 1 embedded sync wait per compute
    instruction (2 for DMA). Tile occasionally emits more; split the excess
    onto same-engine ENGINE_NOPs inserted immediately before the offender
    (safe: no reordering, the nop blocks the engine exactly where the wait
    previously lived)."""
    import bass_rust
    eng = {"DVE": nc.vector, "Activation": nc.scalar, "PE": nc.tensor,
           "Pool": nc.gpsimd, "SP": nc.sync}
    f = nc.m.functions[0]
    for blk in f.blocks:
        il = blk.instructions
        idx = 0
        while idx < len(il):
            ins = il[idx]
            cls = ins.__class__.__name__
            si = ins.sync_info
            if cls == "InstEventSemaphore" or not si or not si.on_wait:
                idx += 1
                continue
            limit = 1
            waits = list(si.on_wait)
            if len(waits) <= limit:
                idx += 1
                continue
            engine_name = str(getattr(ins, "engine", "")).split(".")[-1]
            e = eng.get(engine_name, nc.vector)
            excess = waits[:-limit]
            keep = waits[-limit:]
            upd = list(si.on_update) if si.on_update else []
            ins.sync_info = bass_rust.SyncInfo(on_wait=keep, on_update=upd)
            for w in excess:
                nop = e.nop(nofuse=True)
                mi = nop.ins
                for b2 in f.blocks:
                    l2 = b2.instructions
                    for k in range(len(l2) - 1, -1, -1):
                        if l2[k] is mi:
                            del l2[k]
                mi.sync_info = bass_rust.SyncInfo(on_wait=[w], on_update=[])
                il.insert(idx, mi)
                idx += 1
            idx += 1


def _validate_waits(nc):
    f = nc.m.functions[0]
    bad = []
    for blk in f.blocks:
        for ins in blk.instructions:
            if ins.__class__.__name__ == 'InstEventSemaphore':
                continue
            n = (len(ins.sync_info.on_wait)
                 if ins.sync_info and ins.sync_info.on_wait else 0)
            if n > 1:
                bad.append((ins.name, ins.__class__.__name__, n))
    return bad


def _build_program(ncap, variant=0, pool_adds=POOL_ADDS):
    import concourse.bass as bass
    import concourse.tile as tile
    from concourse import mybir

    f32 = mybir.dt.float32
    f16 = mybir.dt.float16
    bf16 = mybir.dt.bfloat16
    AF = mybir.ActivationFunctionType
    OP = mybir.AluOpType
    AX = mybir.AxisListType

    NB = ncap // 128
    NT = NB // G
    NGRP = NT // GK

    nc = bass.Bass("TRN2", target_bir_lowering=False, debug=False,
                   num_devices=NCORES)

    # xk: per-(tile, partition) contiguous planes [G, 8, 256]; xi: I planes
    xk_d = nc.dram_tensor("xk", [NT * 128, G * 2048], f16,
                          kind="ExternalInput").ap()
    xi_d = nc.dram_tensor("xi", [NT * 128, G * 256], f16,
                          kind="ExternalInput").ap()
    qv_d = nc.dram_tensor("qv", [128, NB], bf16, kind="ExternalInput").ap()
    ohn_d = nc.dram_tensor("ohn", [ncap, 128], bf16, kind="ExternalInput").ap()
    oht_d = nc.dram_tensor("oht", [128, ncap], bf16, kind="ExternalInput").ap()
    w1_d = nc.dram_tensor("w1", [128, 1536], f16, kind="ExternalInput").ap()
    b1_d = nc.dram_tensor("b1", [2, 128], f32, kind="ExternalInput").ap()
    w2_d = nc.dram_tensor("w2", [256, 32], f16, kind="ExternalInput").ap()
    b2_d = nc.dram_tensor("b2", [32, 1], f32, kind="ExternalInput").ap()
    id_d = nc.dram_tensor("ident", [128, 128], f16, kind="ExternalInput").ap()
    idb_d = nc.dram_tensor("identb", [32, 32], bf16, kind="ExternalInput").ap()
    out_d = nc.dram_tensor("out", [ncap, QD], f32, kind="ExternalOutput").ap()

    with tile.TileContext(nc) as tc, ExitStack() as ctx:
        singles = ctx.enter_context(tc.tile_pool(name="singles", bufs=1))
        xp = ctx.enter_context(tc.tile_pool(name="xp", bufs=2))
        fp = ctx.enter_context(tc.tile_pool(name="fp", bufs=1))
        sp = ctx.enter_context(tc.tile_pool(name="sp", bufs=2))
        lt = ctx.enter_context(tc.tile_pool(name="lt", bufs=3))
        ps_mm = ctx.enter_context(tc.tile_pool(name="ps_mm", bufs=3, space="PSUM"))
        ps_t = ctx.enter_context(tc.tile_pool(name="ps_t", bufs=3, space="PSUM"))
        ps_seg = ctx.enter_context(tc.tile_pool(name="ps_seg", bufs=1, space="PSUM"))
        big = ctx.enter_context(tc.tile_pool(name="big", bufs=1))

        # ---- constants / weights / one-hots (loaded once) ----
        ident = singles.tile([128, 128], f16)
        nc.scalar.dma_start(out=ident, in_=id_d)
        identb = singles.tile([32, 32], bf16)
        nc.scalar.dma_start(out=identb, in_=idb_d)
        w1_sb = singles.tile([128, 6, 2, 128], f16)
        nc.scalar.dma_start(out=w1_sb,
                          in_=w1_d.rearrange("p (c jb j) -> p c jb j", c=6, jb=2))
        b1_sb = singles.tile([128, 2], f32)
        nc.scalar.dma_start(out=b1_sb, in_=b1_d.rearrange("c p -> p c"))
        w2_sb = singles.tile([128, 2, 32], f16)
        nc.scalar.dma_start(out=w2_sb, in_=w2_d.rearrange("(c p) q -> p c q", p=128))
        b2c = singles.tile([32, 1], f32)
        nc.scalar.dma_start(out=b2c, in_=b2_d)
        qv_sb = singles.tile([128, NB], bf16)
        nc.scalar.dma_start(out=qv_sb, in_=qv_d)
        u32 = mybir.dt.uint32
        magic = singles.tile([128, GK, G], u32)
        nc.vector.memset(magic, 0x5F3759DF)
        dmy = singles.tile([1, 8], bf16)
        nc.vector.memset(dmy, 0.0)
        nc._legalize_dummy = dmy
        ohn_all = singles.tile([128, NB, 128], bf16)
        nc.gpsimd.dma_start(out=ohn_all,
                            in_=ohn_d.rearrange("(b p) m -> p b m", p=128))
        oht_all = singles.tile([128, NB, 128], bf16)

        # persistent staging across tiles
        cf_st = big.tile([128, NB, 32], bf16)     # [charges | f_u] atom-major
        gath = big.tile([128, NB, 32], bf16)      # gathered [Q_u | recip]
        res = big.tile([128, NB, QD], f32)        # final output staging
        seg_ps = ps_seg.tile([128, 32], f32)      # [Q_u | F_u] per-mol accum

        def phase1(t, fe_tag, ssum, ssq, k):
            xt = xp.tile([128, G, 8, 256], f16, tag="xt")
            nc.sync.dma_start(
                out=xt,
                in_=xk_d[t * 128:(t + 1) * 128].rearrange(
                    "p (g k h) -> p g k h", g=G, h=256))
            fe = fp.tile([128, G, 768], f16, tag=fe_tag)
            nc.sync.dma_start(
                out=fe[:, :, 0:256],
                in_=xi_d[t * 128:(t + 1) * 128].rearrange(
                    "p (g h) -> p g h", g=G))

            # squares in place, split ACT / DVE (GpSimd SBUF traffic stalls
            # concurrent DVE streams ~5x — measured — so Pool stays idle)
            nc.scalar.activation(xt[:, :, 5:8, :], xt[:, :, 5:8, :],
                                 AF.Square)
            nc.vector.tensor_mul(xt[:, :, 0:5, :], xt[:, :, 0:5, :],
                                 xt[:, :, 0:5, :])

            # nA = dd1+dd2+dd3 ; nS = (ss1+ss2+ss3) + (ee0+ee1)
            nA = fe[:, :, 256:512]
            nS = fe[:, :, 512:768]
            scr = sp.tile([128, G, 2, 256], f16, tag="scr")
            nc.vector.tensor_add(nA, xt[:, :, 0, :], xt[:, :, 1, :])
            nc.vector.tensor_add(nA, nA, xt[:, :, 2, :])
            nc.vector.tensor_add(scr[:, :, 0, :], xt[:, :, 3, :], xt[:, :, 4, :])
            nc.vector.tensor_add(scr[:, :, 1, :], xt[:, :, 6, :], xt[:, :, 7, :])
            nc.vector.tensor_add(nS, scr[:, :, 0, :], xt[:, :, 5, :])
            nc.vector.tensor_add(nS, nS, scr[:, :, 1, :])

            # LN stats: Sigma(fe) via pairwise add tree (keeps DVE 2x mode,
            # vs 1x tensor_reduce); Sigma(fe^2) via ACT Square + accumulator
            tr = fp.tile([128, G, 768], f16, tag="tree")
            nc.vector.tensor_add(tr[:, :, 0:384], fe[:, :, 0:384],
                                 fe[:, :, 384:768])
            off, w = 0, 384
            while w > 3:
                h = w // 2
                nc.vector.tensor_add(tr[:, :, off + w:off + w + h],
                                     tr[:, :, off:off + h],
                                     tr[:, :, off + h:off + w])
                off += w
                w = h
            # w == 3 at tr[:, :, off:off+3]
            nc.vector.tensor_add(tr[:, :, off + 3:off + 4],
                                 tr[:, :, off:off + 1],
                                 tr[:, :, off + 1:off + 2])
            nc.vector.tensor_tensor(ssum[:, k], tr[:, :, off + 3:off + 4],
                                    tr[:, :, off + 2:off + 3], OP.add)
            for g in range(G):
                junk = fp.tile([128, 768], f16, tag="junk")
                nc.scalar.activation(junk, fe[:, g, :], AF.Square,
                                     accum_out=ssq[:, k, g:g + 1])
            return fe

        def phase2(t, fe, mu_grp, rstd_grp, k):
            feb = fp.tile([128, G, 768], f16, tag="feb")
            for g in range(G):
                nc.vector.tensor_scalar(feb[:, g, :], fe[:, g, :],
                                        mu_grp[:, k, g:g + 1],
                                        rstd_grp[:, k, g:g + 1],
                                        OP.subtract, OP.mult)

            # transpose ln -> lnT chunks [128f, G*128at]
            lnT = lt.tile([128, 6, G, 128], f16, tag="lnT")
            for cc in range(3):
                tp = ps_t.tile([128, 2, G, 128], f16, tag="tp")
                for ci in range(2):
                    c = 2 * cc + ci
                    for g in range(G):
                        nc.tensor.transpose(
                            tp[:, ci, g, :],
                            feb[:, g, 128 * c:128 * (c + 1)], ident)
                if cc >= 1:
                    nc.scalar.activation(lnT[:, 2 * cc:2 * cc + 2, :, :], tp,
                                         AF.Copy)
                else:
                    nc.vector.tensor_copy(lnT[:, 2 * cc:2 * cc + 2, :, :], tp)

            # mm1 + Silu
            h1T = lt.tile([128, 2, G, 128], f16, tag="h1T")
            for jb in range(2):
                o1 = ps_mm.tile([128, G * 128], f32, tag="mm")
                for c in range(6):
                    nc.tensor.matmul(o1, w1_sb[:, c, jb, :],
                                     lnT[:, c, :, :].rearrange("p g a -> p (g a)"),
                                     start=(c == 0), stop=(c == 5))
                nc.scalar.activation(
                    h1T[:, jb, :, :].rearrange("p g a -> p (g a)"), o1,
                    AF.Silu, bias=b1_sb[:, jb:jb + 1])

            # mm2 (+b2 folded into the PSUM->SBUF copy bias)
            o2 = ps_mm.tile([32, G * 128], f32, tag="mm")
            for c2 in range(2):
                nc.tensor.matmul(o2, w2_sb[:, c2, :],
                                 h1T[:, c2, :, :].rearrange("p g a -> p (g a)"),
                                 start=(c2 == 0), stop=(c2 == 1))
            o2sb = sp.tile([32, G * 128], bf16, tag="o2sb")
            nc.scalar.activation(o2sb, o2, AF.Identity, bias=b2c)

            # atom-major + f_u square + segment accumulate
            pso = ps_t.tile([128, G, 32], bf16, tag="tp")
            for g in range(G):
                nc.tensor.transpose(pso[:, g, :],
                                    o2sb[:, 128 * g:128 * (g + 1)], identb)
            b0 = t * G
            nc.scalar.activation(cf_st[:, b0:b0 + G, 0:16], pso[:, :, 0:16],
                                 AF.Copy)
            nc.scalar.activation(cf_st[:, b0:b0 + G, 16:32], pso[:, :, 16:32],
                                 AF.Square)
            for g in range(G):
                b = b0 + g
                nc.tensor.matmul(seg_ps, ohn_all[:, b, :], cf_st[:, b, :],
                                 start=(b == 0), stop=(b == NB - 1))

        R768 = 1.0 / 768.0
        for tg0 in range(0, NT, GK):
            gksz = min(GK, NT - tg0)
            ssum = sp.tile([128, GK, G, 1], f32, tag="ssum")
            ssq = sp.tile([128, GK, G], f32, tag="ssq")
            mu_grp = sp.tile([128, GK, G], f32, tag="mu")
            rstd_grp = sp.tile([128, GK, G], f32, tag="rstd")
            fes = []
            for k in range(gksz):
                t = tg0 + k
                fes.append(phase1(t, f"fe{t % (GK + 1)}", ssum, ssq, k))
            # mu = ssum/768 ; var = ssq/768 - mu^2 ; rstd = 1/sqrt(var+eps)
            nc.vector.tensor_scalar_mul(mu_grp[:, 0:gksz],
                                        ssum[:, 0:gksz, :, 0], R768)
            nc.vector.tensor_scalar_mul(rstd_grp[:, 0:gksz], ssq[:, 0:gksz],
                                        R768)
            nc.vector.tensor_mul(ssum[:, 0:gksz, :, 0], mu_grp[:, 0:gksz],
                                 mu_grp[:, 0:gksz])
            nc.vector.tensor_tensor(rstd_grp[:, 0:gksz], rstd_grp[:, 0:gksz],
                                    ssum[:, 0:gksz, :, 0], OP.subtract)
            # rstd = rsqrt(var + eps) via fast-inverse-sqrt + 2 Newton steps
            # (all DVE: avoids the ACT Sqrt table switch + engine barrier)
            v = ssq  # scratch: v holds var+eps
            nc.vector.tensor_scalar_add(v[:, 0:gksz], rstd_grp[:, 0:gksz],
                                        LN_EPS)
            vi = rstd_grp.bitcast(u32)
            nc.vector.tensor_scalar(vi[:, 0:gksz], v.bitcast(u32)[:, 0:gksz],
                                    1, None, OP.logical_shift_right)
            nc.vector.tensor_tensor(vi[:, 0:gksz], magic[:, 0:gksz],
                                    vi[:, 0:gksz], OP.subtract)
            y = rstd_grp  # y0 seed now in rstd_grp (as float bits)
            for _ in range(2):
                t2 = ssum[:, :, :, 0]
                nc.vector.tensor_mul(t2[:, 0:gksz], y[:, 0:gksz],
                                     y[:, 0:gksz])
                nc.vector.tensor_mul(t2[:, 0:gksz], t2[:, 0:gksz],
                                     v[:, 0:gksz])
                nc.vector.tensor_scalar(t2[:, 0:gksz], t2[:, 0:gksz],
                                        -0.5, 1.5, OP.mult, OP.add)
                nc.vector.tensor_mul(y[:, 0:gksz], y[:, 0:gksz],
                                     t2[:, 0:gksz])
            for k in range(gksz):
                t = tg0 + k
                phase2(t, fes[k], mu_grp, rstd_grp, k)

        # ---- molecule-level post ----
        nc.gpsimd.dma_start(out=oht_all,
                            in_=oht_d.rearrange("p (b a) -> p b a", a=128))
        mtmp = singles.tile([128, 16], f32)
        nc.vector.tensor_scalar_add(mtmp, seg_ps[:, 16:32], QEQ_EPS)
        nc.vector.reciprocal(mtmp, mtmp)
        mvals = singles.tile([128, 32], bf16)
        nc.vector.tensor_copy(mvals[:, 16:32], mtmp)
        nc.vector.tensor_copy(mvals[:, 0:16], seg_ps[:, 0:16])

        bb = 0
        while bb < NB:
            gw = min(8, NB - bb)
            gp = ps_t.tile([128, 8, 32], f32, tag="tp")
            for j in range(gw):
                nc.tensor.matmul(gp[:, j, :], oht_all[:, bb + j, :], mvals,
                                 start=True, stop=True)
            nc.vector.tensor_copy(gath[:, bb:bb + gw, :], gp[:, 0:gw, :])
            bb += gw

        # ---- batched qeq epilogue ----
        qbc = bass.AP(tensor=qv_sb.tensor, offset=qv_sb.offset,
                      ap=[qv_sb.ap[0], [qv_sb.ap[1][0], NB], [0, QD]])
        # dq = Q - Q_u  (in place over gath Qu slot)
        nc.vector.tensor_tensor(gath[:, :, 0:16], qbc, gath[:, :, 0:16],
                                OP.subtract)
        # scale = f_u * recip (in place over gath recip slot)
        nc.vector.tensor_mul(gath[:, :, 16:32], cf_st[:, :, 16:32],
                             gath[:, :, 16:32])
        corr = xp.tile([128, NB, QD], bf16, tag="xt")
        h = NB // 2
        nc.vector.tensor_mul(corr[:, 0:h], gath[:, 0:h, 0:16],
                             gath[:, 0:h, 16:32])
        nc.vector.tensor_add(res[:, 0:h], cf_st[:, 0:h, 0:16], corr[:, 0:h])
        nc.sync.dma_start(
            out=out_d[0:h * 128].rearrange("(b p) q -> p b q", p=128),
            in_=res[:, 0:h])
        nc.vector.tensor_mul(corr[:, h:NB], gath[:, h:NB, 0:16],
                             gath[:, h:NB, 16:32])
        nc.vector.tensor_add(res[:, h:NB], cf_st[:, h:NB, 0:16],
                             corr[:, h:NB])
        nc.sync.dma_start(
            out=out_d[h * 128:NB * 128].rearrange("(b p) q -> p b q", p=128),
            in_=res[:, h:NB])

    return nc


LAST_EXEC_NS = None


def kernel(X, Q, ln_w, ln_b, W1, b1, W2, b2, batch):
    import ml_dtypes
    from concourse.bass_utils import run_bass_kernel_spmd

    bf = ml_dtypes.bfloat16
    f16 = np.float16
    Xr = np.asarray(X, dtype=np.float32).reshape(N_ATOMS, HID, 9)
    Q = np.asarray(Q, dtype=np.float32)
    batch = np.asarray(batch, dtype=np.int64)

    edges = np.searchsorted(batch, np.arange(0, N_MOL + 1, MPC))
    edges[0] = 0
    edges[-1] = N_ATOMS
    maxcap = int(np.diff(edges).max())
    blk = G * 128
    ncap = max(16896, -(-maxcap // blk) * blk)
    nb = ncap // 128

    # linear orthonormal re-encode: 9 fp16 planes per atom, h-contiguous
    Xp = np.empty((N_ATOMS, 9, HID), dtype=f16)
    Xp[:, 0] = (Xr[:, :, 1] - Xr[:, :, 3]) * SQRT2INV
    Xp[:, 1] = (Xr[:, :, 2] - Xr[:, :, 6]) * SQRT2INV
    Xp[:, 2] = (Xr[:, :, 5] - Xr[:, :, 7]) * SQRT2INV
    Xp[:, 3] = (Xr[:, :, 1] + Xr[:, :, 3]) * SQRT2INV
    Xp[:, 4] = (Xr[:, :, 2] + Xr[:, :, 6]) * SQRT2INV
    Xp[:, 5] = (Xr[:, :, 5] + Xr[:, :, 7]) * SQRT2INV
    Xp[:, 6] = (Xr[:, :, 0] - Xr[:, :, 4]) * SQRT2INV
    Xp[:, 7] = (Xr[:, :, 0] + Xr[:, :, 4] - 2.0 * Xr[:, :, 8]) * SQRT6INV
    Xp[:, 8] = (Xr[:, :, 0] + Xr[:, :, 4] + Xr[:, :, 8]) * (1.0 / 3.0)
    Xp = Xp.reshape(N_ATOMS, 2304)

    ln_w = np.asarray(ln_w, np.float32)
    ln_b = np.asarray(ln_b, np.float32)
    W1 = np.asarray(W1, np.float32)
    W1f = ln_w[:, None] * W1
    b1f = np.asarray(b1, np.float32) + ln_b @ W1
    w1_host = np.ascontiguousarray(
        W1f.reshape(6, 128, 256).transpose(1, 0, 2).reshape(128, 1536)
    ).astype(f16)
    W2h = np.asarray(W2, np.float32).astype(f16)
    b2h = np.ascontiguousarray(
        np.asarray(b2, np.float32).reshape(32, 1))

    nt = nb // G
    in_maps = []
    starts = []
    for c in range(NCORES):
        s, e = int(edges[c]), int(edges[c + 1])
        assert e - s <= ncap, f"core {c} needs {e - s} > {ncap}"
        start = min(s, N_ATOMS - ncap)
        starts.append(start)
        bc = batch[start:start + ncap]
        rel = (bc - c * MPC).astype(np.int64)
        idx = np.arange(ncap) + start
        valid = (idx >= s) & (idx < e) & (rel >= 0) & (rel < MPC)
        ohn = np.zeros((ncap, 128), dtype=np.float32)
        rows = np.nonzero(valid)[0]
        ohn[rows, rel[valid]] = 1.0
        qv = Q[start:start + ncap].reshape(nb, 128).T
        # tile-blocked, partition-contiguous plane/I layout
        v = Xp[start:start + ncap].reshape(nt, G, 128, 2304).transpose(
            0, 2, 1, 3)
        xk = np.ascontiguousarray(v[:, :, :, 0:2048]).reshape(
            nt * 128, G * 2048)
        xi = np.ascontiguousarray(v[:, :, :, 2048:2304]).reshape(
            nt * 128, G * 256)
        in_maps.append({
            "xk": xk,
            "xi": xi,
            "qv": np.ascontiguousarray(qv.astype(bf)),
            "ohn": ohn.astype(bf),
            "oht": np.ascontiguousarray(ohn.T.astype(bf)),
            "w1": w1_host,
            "b1": np.ascontiguousarray(b1f.reshape(2, 128)),
            "w2": W2h,
            "b2": b2h,
            "ident": np.eye(128, dtype=f16),
            "identb": np.eye(32, dtype=bf),
        })

    global LAST_EXEC_NS
    nc = None
    for v in range(2):
        try:
            cand = _build_program(ncap, variant=v)
        except Exception as ex:
            print(f"build variant {v} failed: {ex}")
            continue
        _legalize_waits(cand)
        bad = _validate_waits(cand)
        if not bad:
            nc = cand
            break
        print(f"build variant {v} has over-limit waits: {bad[:3]}")
    assert nc is not None, "no clean build variant found"
    res = run_bass_kernel_spmd(nc, in_maps, core_ids=list(range(NCORES)))
    LAST_EXEC_NS = res.exec_time_ns
    globals()["LAST_RESULT"] = res

    out = np.empty((N_ATOMS, QD), dtype=np.float32)
    for c in range(NCORES):
        s, e = int(edges[c]), int(edges[c + 1])
        r = res.results[c]["out"]
        out[s:e] = r[s - starts[c]:e - starts[c]]
    return out



# revision 29
# speedup vs baseline: 1.0189x; 1.0119x over previous
"""Trainium2 Bass kernel for nn_ChargePredict (segment_reduce).

Sharding: data-parallel over atoms with molecule-aligned shard boundaries so
segment sums stay core-local (one-hot columns zeroed outside each core's own
molecule range; overlap rows discarded on host gather).

The host re-encodes X with a *linear orthonormal* change of basis (same
spirit as the ln_w/ln_b folding): for each (atom, h) the 3x3 block becomes 9
fp16 planes (each 256 h-contiguous)
  [d1,d2,d3, s1,s2,s3, e0,e1, I]
  d_k = (x_ij - x_ji)/sqrt2          (off-diag pairs (0,1),(0,2),(1,2))
  s_k = (x_ij + x_ji)/sqrt2
  e0  = (x00 - x11)/sqrt2,  e1 = (x00 + x11 - 2*x22)/sqrt6
  I   = trace/3
Because (e0, e1) is an orthonormal basis of the traceless-diagonal subspace:
  nA = d1^2+d2^2+d3^2
  nS = s1^2+s2^2+s3^2 + e0^2+e1^2     (no trace correction needed)
  feat = [I, nA, nS] -> LayerNorm -> MLP -> qeq  (identical algebra to ref)
fp16 halves HBM traffic vs fp32 and unlocks DVE 2x modes; squares run mostly
on the Scalar engine, plane sums are 16-bit adds split DVE/GpSimd, LN stats
use bn_stats/bn_aggr, and the LN rsqrt is batched across GK-tile groups so
the ACT table only switches between the silu and sqrt sets twice per group.

Per-core pipeline (atoms on partitions, G=4 blocks of 128 per tile, GK=4
tiles per stats group):
  phase 1 (per tile): DMA planes + I-plane into feat slot; squares in place;
    nA/nS plane adds; bn_stats/bn_aggr
  per group: one Sqrt(var+eps) + reciprocal for GK*G blocks
  phase 2 (per tile): LN apply (TS), PE transposes -> lnT, mm1 fp16 + Silu,
    mm2 fp16 (+b2 via ones-row), out transpose, charges/f^2 (bf16), segment
    matmul with preloaded one-hot blocks
  post: recip(F_u+eps), gather matmuls, batched qeq epilogue
"""

import numpy as np
from contextlib import ExitStack

N_ATOMS = 131072
HID = 256
QD = 16
N_MOL = 1024
LN_EPS = 1e-5
QEQ_EPS = 1e-6

NCORES = 8
MPC = N_MOL // NCORES          # 128 molecules per core
G = 4                          # atom blocks (of 128) per tile
GK = 4                         # tiles per LN-stats group

SQ_DVE = 1                     # planes squared on DVE (rest on ACT)
POOL_ADDS = 0                  # plane-adds offloaded to GpSimd

SQRT2INV = 0.7071067811865476
SQRT6INV = 0.4082482904638631


def _legalize_waits(nc):
    """Walrus codegen accepts at most 1 embedded sync wait per compute
    instruction (2 for DMA). Tile occasionally emits more; split the excess
    onto same-engine ENGINE_NOPs inserted immediately before the offender
    (safe: no reordering, the nop blocks the engine exactly where the wait
    previously lived)."""
    import bass_rust
    eng = {"DVE": nc.vector, "Activation": nc.scalar, "PE": nc.tensor,
           "Pool": nc.gpsimd, "SP": nc.sync}
    f = nc.m.functions[0]
    for blk in f.blocks:
        il = blk.instructions
        idx = 0
        while idx < len(il):
            ins = il[idx]
            cls = ins.__class__.__name__
            si = ins.sync_info
            if cls == "InstEventSemaphore" or not si or not si.on_wait:
                idx += 1
                continue
            limit = 1
            waits = list(si.on_wait)
            if len(waits) <= limit:
                idx += 1
                continue
            engine_name = str(getattr(ins, "engine", "")).split(".")[-1]
            e = eng.get(engine_name, nc.vector)
            excess = waits[:-limit]
            keep = waits[-limit:]
            upd = list(si.on_update) if si.on_update else []
            ins.sync_info = bass_rust.SyncInfo(on_wait=keep, on_update=upd)
            for w in excess:
                nop = e.nop(nofuse=True)
                mi = nop.ins
                for b2 in f.blocks:
                    l2 = b2.instructions
                    for k in range(len(l2) - 1, -1, -1):
                        if l2[k] is mi:
                            del l2[k]
                mi.sync_info = bass_rust.SyncInfo(on_wait=[w], on_update=[])
                il.insert(idx, mi)
                idx += 1
            idx += 1


def _validate_waits(nc):
    f = nc.m.functions[0]
    bad = []
    for blk in f.blocks:
        for ins in blk.instructions:
            if ins.__class__.__name__ == 'InstEventSemaphore':
                continue
            n = (len(ins.sync_info.on_wait)
                 if ins.sync_info and ins.sync_info.on_wait else 0)
            if n > 1:
                bad.append((ins.name, ins.__class__.__name__, n))
    return bad


def _build_program(ncap, variant=0, pool_adds=POOL_ADDS):
    import concourse.bass as bass
    import concourse.tile as tile
    from concourse import mybir

    f32 = mybir.dt.float32
    f16 = mybir.dt.float16
    bf16 = mybir.dt.bfloat16
    AF = mybir.ActivationFunctionType
    OP = mybir.AluOpType
    AX = mybir.AxisListType

    NB = ncap // 128
    NT = NB // G
    NGRP = NT // GK

    nc = bass.Bass("TRN2", target_bir_lowering=False, debug=False,
                   num_devices=NCORES)

    # xk: per-(tile, partition) contiguous planes [G, 8, 256]; xi: I planes
    xk_d = nc.dram_tensor("xk", [NT * 128, G * 2048], f16,
                          kind="ExternalInput").ap()
    xi_d = nc.dram_tensor("xi", [NT * 128, G * 256], f16,
                          kind="ExternalInput").ap()
    qv_d = nc.dram_tensor("qv", [128, NB], bf16, kind="ExternalInput").ap()
    ohn_d = nc.dram_tensor("ohn", [ncap, 128], bf16, kind="ExternalInput").ap()
    oht_d = nc.dram_tensor("oht", [128, ncap], bf16, kind="ExternalInput").ap()
    w1_d = nc.dram_tensor("w1", [128, 1536], f16, kind="ExternalInput").ap()
    w1c_d = nc.dram_tensor("w1c", [1, 256], f16, kind="ExternalInput").ap()
    b1_d = nc.dram_tensor("b1", [2, 128], f32, kind="ExternalInput").ap()
    w2_d = nc.dram_tensor("w2", [256, 32], f16, kind="ExternalInput").ap()
    b2_d = nc.dram_tensor("b2", [32, 1], f32, kind="ExternalInput").ap()
    id_d = nc.dram_tensor("ident", [128, 128], f16, kind="ExternalInput").ap()
    idb_d = nc.dram_tensor("identb", [32, 32], bf16, kind="ExternalInput").ap()
    out_d = nc.dram_tensor("out", [ncap, QD], f32, kind="ExternalOutput").ap()

    with tile.TileContext(nc) as tc, ExitStack() as ctx:
        singles = ctx.enter_context(tc.tile_pool(name="singles", bufs=1))
        xp = ctx.enter_context(tc.tile_pool(name="xp", bufs=2))
        fp = ctx.enter_context(tc.tile_pool(name="fp", bufs=1))
        sp = ctx.enter_context(tc.tile_pool(name="sp", bufs=2))
        lt = ctx.enter_context(tc.tile_pool(name="lt", bufs=3))
        ps_mm = ctx.enter_context(tc.tile_pool(name="ps_mm", bufs=3, space="PSUM"))
        ps_t = ctx.enter_context(tc.tile_pool(name="ps_t", bufs=3, space="PSUM"))
        ps_seg = ctx.enter_context(tc.tile_pool(name="ps_seg", bufs=1, space="PSUM"))
        big = ctx.enter_context(tc.tile_pool(name="big", bufs=1))

        # ---- constants / weights / one-hots (loaded once) ----
        ident = singles.tile([128, 128], f16)
        nc.scalar.dma_start(out=ident, in_=id_d)
        identb = singles.tile([32, 32], bf16)
        nc.scalar.dma_start(out=identb, in_=idb_d)
        w1_sb = singles.tile([128, 6, 2, 128], f16)
        nc.scalar.dma_start(out=w1_sb,
                          in_=w1_d.rearrange("p (c jb j) -> p c jb j", c=6, jb=2))
        b1_sb = singles.tile([128, 2], f32)
        nc.scalar.dma_start(out=b1_sb, in_=b1_d.rearrange("c p -> p c"))
        w2_sb = singles.tile([128, 2, 32], f16)
        nc.scalar.dma_start(out=w2_sb, in_=w2_d.rearrange("(c p) q -> p c q", p=128))
        b2c = singles.tile([32, 1], f32)
        nc.scalar.dma_start(out=b2c, in_=b2_d)
        w1c = singles.tile([1, 2, 128], f16)
        nc.scalar.dma_start(out=w1c, in_=w1c_d.rearrange("x (c j) -> x c j", c=2))
        qv_sb = singles.tile([128, NB], bf16)
        nc.scalar.dma_start(out=qv_sb, in_=qv_d)
        u32 = mybir.dt.uint32
        magic = singles.tile([128, GK, G], u32)
        nc.vector.memset(magic, 0x5F3759DF)
        dmy = singles.tile([1, 8], bf16)
        nc.vector.memset(dmy, 0.0)
        nc._legalize_dummy = dmy
        ohn_all = singles.tile([128, NB, 128], bf16)
        ohn_r = ohn_d.rearrange("(b p) m -> p b m", p=128)
        hb = NB // 2
        nc.gpsimd.dma_start(out=ohn_all[:, 0:hb], in_=ohn_r[:, 0:hb])
        nc.gpsimd.dma_start(out=ohn_all[:, hb:NB], in_=ohn_r[:, hb:NB])
        oht_all = singles.tile([128, NB, 128], bf16)

        # persistent staging across tiles
        cf_st = big.tile([128, NB, 32], bf16)     # [charges | f_u] atom-major
        gath = big.tile([128, NB, 32], bf16)      # gathered [Q_u | recip]
        res = big.tile([128, NB, QD], f32)        # final output staging
        seg_ps = ps_seg.tile([128, 32], f32)      # [Q_u | F_u] per-mol accum

        def phase1(t, fe_tag, ssum, ssq, k):
            xt = xp.tile([128, G, 8, 256], f16, tag="xt")
            nc.sync.dma_start(
                out=xt,
                in_=xk_d[t * 128:(t + 1) * 128].rearrange(
                    "p (g k h) -> p g k h", g=G, h=256))
            fe = fp.tile([128, G, 768], f16, tag=fe_tag)
            nc.sync.dma_start(
                out=fe[:, :, 0:256],
                in_=xi_d[t * 128:(t + 1) * 128].rearrange(
                    "p (g h) -> p g h", g=G))

            # squares in place, split ACT / DVE (GpSimd SBUF traffic stalls
            # concurrent DVE streams ~5x — measured — so Pool stays idle)
            nc.scalar.activation(xt[:, :, 5:8, :], xt[:, :, 5:8, :],
                                 AF.Square)
            nc.vector.tensor_mul(xt[:, :, 0:5, :], xt[:, :, 0:5, :],
                                 xt[:, :, 0:5, :])

            # nA = dd1+dd2+dd3 ; nS = (ss1+ss2+ss3) + (ee0+ee1)
            nA = fe[:, :, 256:512]
            nS = fe[:, :, 512:768]
            scr = sp.tile([128, G, 2, 256], f16, tag="scr")
            nc.vector.tensor_add(nA, xt[:, :, 0, :], xt[:, :, 1, :])
            nc.vector.tensor_add(nA, nA, xt[:, :, 2, :])
            nc.vector.tensor_add(scr[:, :, 0, :], xt[:, :, 3, :], xt[:, :, 4, :])
            nc.vector.tensor_add(scr[:, :, 1, :], xt[:, :, 6, :], xt[:, :, 7, :])
            nc.vector.tensor_add(nS, scr[:, :, 0, :], xt[:, :, 5, :])
            nc.vector.tensor_add(nS, nS, scr[:, :, 1, :])

            # LN stats: Sigma(fe) via pairwise add tree (keeps DVE 2x mode,
            # vs 1x tensor_reduce); Sigma(fe^2) via ACT Square + accumulator
            tr = fp.tile([128, G, 768], f16, tag="tree")
            nc.vector.tensor_add(tr[:, :, 0:384], fe[:, :, 0:384],
                                 fe[:, :, 384:768])
            off, w = 0, 384
            while w > 3:
                h = w // 2
                nc.vector.tensor_add(tr[:, :, off + w:off + w + h],
                                     tr[:, :, off:off + h],
                                     tr[:, :, off + h:off + w])
                off += w
                w = h
            # w == 3 at tr[:, :, off:off+3]
            nc.vector.tensor_add(tr[:, :, off + 3:off + 4],
                                 tr[:, :, off:off + 1],
                                 tr[:, :, off + 1:off + 2])
            nc.vector.tensor_tensor(ssum[:, k], tr[:, :, off + 3:off + 4],
                                    tr[:, :, off + 2:off + 3], OP.add)
            for g in range(G):
                junk = fp.tile([128, 768], f16, tag="junk")
                nc.scalar.activation(junk, fe[:, g, :], AF.Square,
                                     accum_out=ssq[:, k, g:g + 1])
            return fe

        def phase2(t, fe, mu_grp, rstd_grp, k):
            # LN apply folded into PE: transposes scale by diag(rstd); the
            # -mu*rstd*colsum(W1) rank-1 term accumulates into mm1 below
            dg = sp.tile([128, G, 128], f16, tag="diag")
            for g in range(G):
                nc.vector.tensor_scalar(dg[:, g, :], ident,
                                        rstd_grp[:, k, g:g + 1], None,
                                        OP.mult)
            mur = sp.tile([128, G], f16, tag="mur")
            nc.vector.tensor_tensor(mur, mu_grp[:, k], rstd_grp[:, k],
                                    OP.mult)
            pt = ps_t.tile([128, 128], f16, tag="mt")
            nc.tensor.transpose(pt[0:G, :], mur, ident)
            murT = sp.tile([G, 128], f16, tag="murT")
            nc.vector.tensor_copy(murT, pt[0:G, :])

            # transpose rstd*fe -> lnT chunks [128f, G*128at]
            lnT = lt.tile([128, 6, G, 128], f16, tag="lnT")
            for cc in range(3):
                tp = ps_t.tile([128, 2, G, 128], f16, tag="tp")
                for ci in range(2):
                    c = 2 * cc + ci
                    for g in range(G):
                        nc.tensor.transpose(
                            tp[:, ci, g, :],
                            fe[:, g, 128 * c:128 * (c + 1)], dg[:, g, :])
                if cc >= 1:
                    nc.scalar.activation(lnT[:, 2 * cc:2 * cc + 2, :, :], tp,
                                         AF.Copy)
                else:
                    nc.vector.tensor_copy(lnT[:, 2 * cc:2 * cc + 2, :, :], tp)

            # mm1 + Silu
            h1T = lt.tile([128, 2, G, 128], f16, tag="h1T")
            for jb in range(2):
                o1 = ps_mm.tile([128, G * 128], f32, tag="mm")
                for c in range(6):
                    nc.tensor.matmul(o1, w1_sb[:, c, jb, :],
                                     lnT[:, c, :, :].rearrange("p g a -> p (g a)"),
                                     start=(c == 0), stop=False)
                for g in range(G):
                    nc.tensor.matmul(o1[:, 128 * g:128 * (g + 1)],
                                     w1c[:, jb, :], murT[g:g + 1, :],
                                     start=False, stop=(g == G - 1))
                nc.scalar.activation(
                    h1T[:, jb, :, :].rearrange("p g a -> p (g a)"), o1,
                    AF.Silu, bias=b1_sb[:, jb:jb + 1])

            # mm2 (+b2 folded into the PSUM->SBUF copy bias)
            o2 = ps_mm.tile([32, G * 128], f32, tag="mm")
            for c2 in range(2):
                nc.tensor.matmul(o2, w2_sb[:, c2, :],
                                 h1T[:, c2, :, :].rearrange("p g a -> p (g a)"),
                                 start=(c2 == 0), stop=(c2 == 1))
            o2sb = sp.tile([32, G * 128], bf16, tag="o2sb")
            nc.scalar.activation(o2sb, o2, AF.Identity, bias=b2c)

            # atom-major + f_u square + segment accumulate
            pso = ps_t.tile([128, G, 32], bf16, tag="tp")
            for g in range(G):
                nc.tensor.transpose(pso[:, g, :],
                                    o2sb[:, 128 * g:128 * (g + 1)], identb)
            b0 = t * G
            nc.scalar.activation(cf_st[:, b0:b0 + G, 0:16], pso[:, :, 0:16],
                                 AF.Copy)
            nc.scalar.activation(cf_st[:, b0:b0 + G, 16:32], pso[:, :, 16:32],
                                 AF.Square)
            for g in range(G):
                b = b0 + g
                nc.tensor.matmul(seg_ps, ohn_all[:, b, :], cf_st[:, b, :],
                                 start=(b == 0), stop=(b == NB - 1))

        R768 = 1.0 / 768.0
        for tg0 in range(0, NT, GK):
            gksz = min(GK, NT - tg0)
            ssum = sp.tile([128, GK, G, 1], f32, tag="ssum")
            ssq = sp.tile([128, GK, G], f32, tag="ssq")
            mu_grp = sp.tile([128, GK, G], f32, tag="mu")
            rstd_grp = sp.tile([128, GK, G], f32, tag="rstd")
            fes = []
            for k in range(gksz):
                t = tg0 + k
                fes.append(phase1(t, f"fe{t % (GK + 1)}", ssum, ssq, k))
            # mu = ssum/768 ; var = ssq/768 - mu^2 ; rstd = 1/sqrt(var+eps)
            nc.vector.tensor_scalar_mul(mu_grp[:, 0:gksz],
                                        ssum[:, 0:gksz, :, 0], R768)
            nc.vector.tensor_scalar_mul(rstd_grp[:, 0:gksz], ssq[:, 0:gksz],
                                        R768)
            nc.vector.tensor_mul(ssum[:, 0:gksz, :, 0], mu_grp[:, 0:gksz],
                                 mu_grp[:, 0:gksz])
            nc.vector.tensor_tensor(rstd_grp[:, 0:gksz], rstd_grp[:, 0:gksz],
                                    ssum[:, 0:gksz, :, 0], OP.subtract)
            # rstd = rsqrt(var + eps) via fast-inverse-sqrt + 2 Newton steps
            # (all DVE: avoids the ACT Sqrt table switch + engine barrier)
            v = ssq  # scratch: v holds var+eps
            nc.vector.tensor_scalar_add(v[:, 0:gksz], rstd_grp[:, 0:gksz],
                                        LN_EPS)
            vi = rstd_grp.bitcast(u32)
            nc.vector.tensor_scalar(vi[:, 0:gksz], v.bitcast(u32)[:, 0:gksz],
                                    1, None, OP.logical_shift_right)
            nc.vector.tensor_tensor(vi[:, 0:gksz], magic[:, 0:gksz],
                                    vi[:, 0:gksz], OP.subtract)
            y = rstd_grp  # y0 seed now in rstd_grp (as float bits)
            for _ in range(2):
                t2 = ssum[:, :, :, 0]
                nc.vector.tensor_mul(t2[:, 0:gksz], y[:, 0:gksz],
                                     y[:, 0:gksz])
                nc.vector.tensor_mul(t2[:, 0:gksz], t2[:, 0:gksz],
                                     v[:, 0:gksz])
                nc.vector.tensor_scalar(t2[:, 0:gksz], t2[:, 0:gksz],
                                        -0.5, 1.5, OP.mult, OP.add)
                nc.vector.tensor_mul(y[:, 0:gksz], y[:, 0:gksz],
                                     t2[:, 0:gksz])
            for k in range(gksz):
                t = tg0 + k
                phase2(t, fes[k], mu_grp, rstd_grp, k)

        # ---- molecule-level post ----
        oht_r = oht_d.rearrange("p (b a) -> p b a", a=128)
        nc.gpsimd.dma_start(out=oht_all[:, 0:hb], in_=oht_r[:, 0:hb])
        nc.gpsimd.dma_start(out=oht_all[:, hb:NB], in_=oht_r[:, hb:NB])
        mtmp = singles.tile([128, 16], f32)
        nc.vector.tensor_scalar_add(mtmp, seg_ps[:, 16:32], QEQ_EPS)
        nc.vector.reciprocal(mtmp, mtmp)
        mvals = singles.tile([128, 32], bf16)
        nc.vector.tensor_copy(mvals[:, 16:32], mtmp)
        nc.vector.tensor_copy(mvals[:, 0:16], seg_ps[:, 0:16])

        bb = 0
        while bb < NB:
            gw = min(8, NB - bb)
            gp = ps_t.tile([128, 8, 32], f32, tag="tp")
            for j in range(gw):
                nc.tensor.matmul(gp[:, j, :], oht_all[:, bb + j, :], mvals,
                                 start=True, stop=True)
            nc.vector.tensor_copy(gath[:, bb:bb + gw, :], gp[:, 0:gw, :])
            bb += gw

        # ---- batched qeq epilogue ----
        qbc = bass.AP(tensor=qv_sb.tensor, offset=qv_sb.offset,
                      ap=[qv_sb.ap[0], [qv_sb.ap[1][0], NB], [0, QD]])
        # dq = Q - Q_u  (in place over gath Qu slot)
        nc.vector.tensor_tensor(gath[:, :, 0:16], qbc, gath[:, :, 0:16],
                                OP.subtract)
        # scale = f_u * recip (in place over gath recip slot)
        nc.vector.tensor_mul(gath[:, :, 16:32], cf_st[:, :, 16:32],
                             gath[:, :, 16:32])
        corr = xp.tile([128, NB, QD], bf16, tag="xt")
        h = NB // 2
        nc.vector.tensor_mul(corr[:, 0:h], gath[:, 0:h, 0:16],
                             gath[:, 0:h, 16:32])
        nc.vector.tensor_add(res[:, 0:h], cf_st[:, 0:h, 0:16], corr[:, 0:h])
        nc.sync.dma_start(
            out=out_d[0:h * 128].rearrange("(b p) q -> p b q", p=128),
            in_=res[:, 0:h])
        nc.vector.tensor_mul(corr[:, h:NB], gath[:, h:NB, 0:16],
                             gath[:, h:NB, 16:32])
        nc.vector.tensor_add(res[:, h:NB], cf_st[:, h:NB, 0:16],
                             corr[:, h:NB])
        nc.sync.dma_start(
            out=out_d[h * 128:NB * 128].rearrange("(b p) q -> p b q", p=128),
            in_=res[:, h:NB])

    return nc


LAST_EXEC_NS = None


def kernel(X, Q, ln_w, ln_b, W1, b1, W2, b2, batch):
    import ml_dtypes
    from concourse.bass_utils import run_bass_kernel_spmd

    bf = ml_dtypes.bfloat16
    f16 = np.float16
    Xr = np.asarray(X, dtype=np.float32).reshape(N_ATOMS, HID, 9)
    Q = np.asarray(Q, dtype=np.float32)
    batch = np.asarray(batch, dtype=np.int64)

    edges = np.searchsorted(batch, np.arange(0, N_MOL + 1, MPC))
    edges[0] = 0
    edges[-1] = N_ATOMS
    maxcap = int(np.diff(edges).max())
    blk = G * 128
    ncap = max(16896, -(-maxcap // blk) * blk)
    nb = ncap // 128

    # linear orthonormal re-encode: 9 fp16 planes per atom, h-contiguous
    Xp = np.empty((N_ATOMS, 9, HID), dtype=f16)
    Xp[:, 0] = (Xr[:, :, 1] - Xr[:, :, 3]) * SQRT2INV
    Xp[:, 1] = (Xr[:, :, 2] - Xr[:, :, 6]) * SQRT2INV
    Xp[:, 2] = (Xr[:, :, 5] - Xr[:, :, 7]) * SQRT2INV
    Xp[:, 3] = (Xr[:, :, 1] + Xr[:, :, 3]) * SQRT2INV
    Xp[:, 4] = (Xr[:, :, 2] + Xr[:, :, 6]) * SQRT2INV
    Xp[:, 5] = (Xr[:, :, 5] + Xr[:, :, 7]) * SQRT2INV
    Xp[:, 6] = (Xr[:, :, 0] - Xr[:, :, 4]) * SQRT2INV
    Xp[:, 7] = (Xr[:, :, 0] + Xr[:, :, 4] - 2.0 * Xr[:, :, 8]) * SQRT6INV
    Xp[:, 8] = (Xr[:, :, 0] + Xr[:, :, 4] + Xr[:, :, 8]) * (1.0 / 3.0)
    Xp = Xp.reshape(N_ATOMS, 2304)

    ln_w = np.asarray(ln_w, np.float32)
    ln_b = np.asarray(ln_b, np.float32)
    W1 = np.asarray(W1, np.float32)
    W1f = ln_w[:, None] * W1
    b1f = np.asarray(b1, np.float32) + ln_b @ W1
    w1_host = np.ascontiguousarray(
        W1f.reshape(6, 128, 256).transpose(1, 0, 2).reshape(128, 1536)
    ).astype(f16)
    W2h = np.asarray(W2, np.float32).astype(f16)
    b2h = np.ascontiguousarray(
        np.asarray(b2, np.float32).reshape(32, 1))

    nt = nb // G
    in_maps = []
    starts = []
    for c in range(NCORES):
        s, e = int(edges[c]), int(edges[c + 1])
        assert e - s <= ncap, f"core {c} needs {e - s} > {ncap}"
        start = min(s, N_ATOMS - ncap)
        starts.append(start)
        bc = batch[start:start + ncap]
        rel = (bc - c * MPC).astype(np.int64)
        idx = np.arange(ncap) + start
        valid = (idx >= s) & (idx < e) & (rel >= 0) & (rel < MPC)
        ohn = np.zeros((ncap, 128), dtype=np.float32)
        rows = np.nonzero(valid)[0]
        ohn[rows, rel[valid]] = 1.0
        qv = Q[start:start + ncap].reshape(nb, 128).T
        # tile-blocked, partition-contiguous plane/I layout
        v = Xp[start:start + ncap].reshape(nt, G, 128, 2304).transpose(
            0, 2, 1, 3)
        xk = np.ascontiguousarray(v[:, :, :, 0:2048]).reshape(
            nt * 128, G * 2048)
        xi = np.ascontiguousarray(v[:, :, :, 2048:2304]).reshape(
            nt * 128, G * 256)
        in_maps.append({
            "xk": xk,
            "xi": xi,
            "qv": np.ascontiguousarray(qv.astype(bf)),
            "ohn": ohn.astype(bf),
            "oht": np.ascontiguousarray(ohn.T.astype(bf)),
            "w1": w1_host,
            "b1": np.ascontiguousarray(b1f.reshape(2, 128)),
            "w2": W2h,
            "b2": b2h,
            "ident": np.eye(128, dtype=f16),
            "identb": np.eye(32, dtype=bf),
        })

    global LAST_EXEC_NS
    nc = None
    for v in range(2):
        try:
            cand = _build_program(ncap, variant=v)
        except Exception as ex:
            print(f"build variant {v} failed: {ex}")
            continue
        _legalize_waits(cand)
        bad = _validate_waits(cand)
        if not bad:
            nc = cand
            break
        print(f"build variant {v} has over-limit waits: {bad[:3]}")
    assert nc is not None, "no clean build variant found"
    res = run_bass_kernel_spmd(nc, in_maps, core_ids=list(range(NCORES)))
    LAST_EXEC_NS = res.exec_time_ns
    globals()["LAST_RESULT"] = res

    out = np.empty((N_ATOMS, QD), dtype=np.float32)
    for c in range(NCORES):
        s, e = int(edges[c]), int(edges[c + 1])
        r = res.results[c]["out"]
        out[s:e] = r[s - starts[c]:e - starts[c]]
    return out

